# revision 4
# baseline (speedup 1.0000x reference)
"""CFConvS2V Trainium2 kernel (8-core data-parallel over batch), v2.

reference computation:
    h = silu(layernorm(s @ W1.T + b1))               # (B, N, H)
    v[b,i,c,d] = sum_j mask[b,i,j] * ev[b,i,j,c] * h[b,j,d]   # (B, N, 3, H)

Sharding: data-parallel over B across 8 cores (4 batches each); the pairwise
tensors and the j-reduction stay local per core.

v2 design (memory-roofline targeted):
  * All big tensors staged in fp16 (host-side cast): halves HBM traffic vs
    fp32. Element rounding ~5e-4 relative, far inside the 2e-2 gate.
  * ev is staged TRANSPOSED on the host to [b, j, (c, i)] and concatenated
    with the transposed mask [b, j, i] into one [BL, N, 2048] tensor: one
    contiguous 512KB DMA per (batch, j-chunk), and j lands on partitions so
    the contraction needs NO on-chip transposes (the baseline spent ~30us of
    PE time on 192 transposes/core).
  * mask applied by one DVE multiply per j-chunk in [j, (c, i)] layout
    (mask broadcast over c via a 0-stride AP); fp16 keeps DVE in 2x mode.
  * contraction: per (b, jc): 4 accumulating fp16 matmuls with h[jc]
    stationary and mev [j, (c, i-tile)] moving into 4 PSUM accumulators
    (one per i-tile, 384 cols each).
  * h-phase: bias folded into the matmul as a K=1 rank-1 update (ones (x) b1)
    so no DVE bias pass; LayerNorm stats straight from PSUM via one
    multi-chunk bn_stats; SiLU fused on ACT (func=Silu) emitting fp16 h.
  * output evicted to fp16 and stored as [d, (it, c, il)]; host reorders and
    upcasts. Total HBM traffic/core ~10.6 MB -> ~30us roofline at 358 GB/s.
"""

import sys

if "/opt/trn_rl_repo" not in sys.path:
    sys.path.insert(0, "/opt/trn_rl_repo")

from contextlib import ExitStack

import numpy as np

import concourse.bass as bass
import concourse.mybir as mybir
from concourse.tile import TileContext

B, N, H, C = 32, 512, 128, 3
NCORES = 8
BL = B // NCORES      # batches per core
P = 128
NT = N // P           # i-tiles per batch
JC = N // P           # j-chunks
EVW = C * N + N       # ev row (1536) + mask row (512) per (b, j)
LN_EPS = 1e-5
F32 = mybir.dt.float32
F16 = mybir.dt.float16
AF = mybir.ActivationFunctionType
ALU = mybir.AluOpType


def _split_multi_waits(nc):
    """The walrus build in this container only accepts one sync-wait per
    instruction; hoist extra waits onto single-wait NOPs in front."""
    ctr = 0
    for f in nc.m.functions:
        for bb in f.blocks:
            insts = bb.instructions
            i = 0
            while i < len(insts):
                inst = insts[i]
                si = inst.sync_info
                if si is not None and len(si.on_wait) > 1:
                    waits = list(si.on_wait)
                    for w in waits[:-1]:
                        ctr += 1
                        nop = mybir.InstNoOp(
                            name=f"splitwait-{ctr}",
                            engine=inst.engine,
                            sync_info=mybir.SyncInfo(on_wait=[w], on_update=[]),
                            bass_nofuse=True,
                        )
                        nc.register_instruction(nop, overwrite=True)
                        insts.insert(i, nop)
                        i += 1
                    inst.sync_info = mybir.SyncInfo(
                        on_wait=[waits[-1]], on_update=list(si.on_update)
                    )
                i += 1


def build(reps=1):
    nc = bass.Bass("TRN2", target_bir_lowering=False, debug=False, num_devices=NCORES)
    evm = nc.dram_tensor("evm", [BL, N, EVW], F16, kind="ExternalInput").ap()
    sT = nc.dram_tensor("sT", [BL, H, N], F16, kind="ExternalInput").ap()
    w1t = nc.dram_tensor("w1t", [H, H], F16, kind="ExternalInput").ap()
    # cst row: [ones(H) | tile(b1, NT)] for the K=1 bias matmul
    cst = nc.dram_tensor("cst", [1, H + N], F16, kind="ExternalInput").ap()
    out = nc.dram_tensor("out", [BL, H, NT * C * P], F16, kind="ExternalOutput").ap()

    with TileContext(nc) as tc, ExitStack() as ctx:
        const = ctx.enter_context(tc.tile_pool(name="const", bufs=1))
        p_sT = ctx.enter_context(tc.tile_pool(name="p_sT", bufs=2))
        p_h = ctx.enter_context(tc.tile_pool(name="p_h", bufs=2))
        p_xn = ctx.enter_context(tc.tile_pool(name="p_xn", bufs=2))
        p_stat = ctx.enter_context(tc.tile_pool(name="p_stat", bufs=4))
        p_evm = ctx.enter_context(tc.tile_pool(name="p_evm", bufs=4))
        p_mev = ctx.enter_context(tc.tile_pool(name="p_mev", bufs=4))
        p_vout = ctx.enter_context(tc.tile_pool(name="p_vout", bufs=3))
        ps_h = ctx.enter_context(tc.tile_pool(name="ps_h", bufs=2, space="PSUM"))
        ps_v = ctx.enter_context(tc.tile_pool(name="ps_v", bufs=6, space="PSUM"))

        w1t_sb = const.tile([H, H], F16)
        nc.sync.dma_start(out=w1t_sb[:], in_=w1t[:])
        cst_sb = const.tile([1, H + N], F16)
        nc.sync.dma_start(out=cst_sb[:], in_=cst[:])
        eps_sb = const.tile([P, 1], F32)
        nc.vector.memset(eps_sb[:], LN_EPS)

        def body():
            # ---------- h phase (all batches): h = silu(LN(s @ W1.T + b1)) ----
            sT_sb = p_sT.tile([P, BL, N], F16)
            for b in range(BL):
                nc.sync.dma_start(out=sT_sb[:, b, :], in_=sT[b])
            h_sb = p_h.tile([P, BL, N], F16)
            for b in range(BL):
                psum_h = ps_h.tile([P, N], F32)
                # rank-1 bias: ones(x)b1 seeds the accumulator
                nc.tensor.matmul(
                    out=psum_h[:],
                    lhsT=cst_sb[:, 0:H],
                    rhs=cst_sb[:, H:],
                    start=True,
                    stop=False,
                    skip_group_check=True,
                )
                for t in range(NT):
                    # out[n_local, k] += sum_h sT[h, n] * W1T[h, k]
                    nc.tensor.matmul(
                        out=psum_h[:, t * P : (t + 1) * P],
                        lhsT=sT_sb[:, b, t * P : (t + 1) * P],
                        rhs=w1t_sb[:],
                        start=False,
                        stop=(t == NT - 1),
                        skip_group_check=True,
                    )
                mv = p_stat.tile([P, NT, 2], F32, tag="mv")
                for t in range(NT):
                    stats = p_stat.tile([P, 6], F32, tag="stats")
                    nc.vector.bn_stats(
                        out=stats[:], in_=psum_h[:, t * P : (t + 1) * P]
                    )
                    nc.vector.bn_aggr(out=mv[:, t, :], in_=stats[:])
                rstd = p_stat.tile([P, NT, 1], F32, tag="rstd")
                nc.scalar.activation(
                    out=rstd[:], in_=mv[:, :, 1:2], func=AF.Sqrt, bias=eps_sb[:]
                )
                nc.vector.reciprocal(out=rstd[:], in_=rstd[:])
                xn = p_xn.tile([P, NT, H], F32)
                for t in range(NT):
                    nc.vector.tensor_scalar(
                        out=xn[:, t, :],
                        in0=psum_h[:, t * P : (t + 1) * P],
                        scalar1=mv[:, t, 0:1],
                        scalar2=rstd[:, t, 0:1],
                        op0=ALU.subtract,
                        op1=ALU.mult,
                    )
                nc.scalar.activation(
                    out=h_sb[:, b, :],
                    in_=xn[:].rearrange("p t k -> p (t k)"),
                    func=AF.Silu,
                )

            # ---------- main phase: v[d,(it,c,il)] = sum_j h[j,d]*mev[j,(c,i)] --
            for b in range(BL):
                psvs = [
                    ps_v.tile([P, 512], F32, name=f"psv{it}", tag="psv")
                    for it in range(NT)
                ]
                for jc in range(JC):
                    evm_sb = p_evm.tile([P, EVW], F16)
                    nc.sync.dma_start(
                        out=evm_sb[:], in_=evm[b, jc * P : (jc + 1) * P]
                    )
                    # mev[j,(c,i)] = ev[j,(c,i)] * mask[j,i]  (broadcast over c)
                    mev = p_mev.tile([P, C, N], F16)
                    nc.vector.tensor_tensor(
                        out=mev[:],
                        in0=evm_sb[:, : C * N].rearrange("p (c i) -> p c i", i=N),
                        in1=evm_sb[:, C * N :].unsqueeze(1).broadcast_to((P, C, N)),
                        op=ALU.mult,
                    )
                    for it in range(NT):
                        # v[d, (c,il)] += sum_j h[j, d] * mev[j, (c, il)]
                        nc.tensor.matmul(
                            out=psvs[it][:, : C * P],
                            lhsT=h_sb[:, b, jc * P : (jc + 1) * P],
                            rhs=mev[:, :, it * P : (it + 1) * P],
                            start=(jc == 0),
                            stop=(jc == JC - 1),
                            skip_group_check=True,
                        )
                vout = p_vout.tile([P, NT, C * P], F16)
                for it in range(NT):
                    nc.scalar.activation(
                        out=vout[:, it, :], in_=psvs[it][:, : C * P], func=AF.Copy
                    )
                # store on the ACT HWDGE ring so stores can't block loads on
                # the SP-ring FIFO
                nc.scalar.dma_start(
                    out=out[b], in_=vout[:].rearrange("p t f -> p (t f)")
                )

        if reps == 1:
            body()
        else:
            with tc.For_i(0, reps, 1):
                body()

    _split_multi_waits(nc)
    return nc


_built_nc = None


def _get_nc():
    global _built_nc
    if _built_nc is None:
        _built_nc = build()
    return _built_nc


def shard_inputs(s, ev, mask, W1, b1):
    """Full inputs -> list of per-core input dicts (fp16 staged layouts)."""
    s = np.asarray(s, dtype=np.float32)
    ev = np.asarray(ev, dtype=np.float32)
    mask = np.asarray(mask, dtype=np.float32)
    W1 = np.asarray(W1, dtype=np.float32)
    b1 = np.asarray(b1, dtype=np.float32)
    w1t = np.ascontiguousarray(W1.T).astype(np.float16)
    cst = np.concatenate(
        [np.ones((1, H), np.float32), np.tile(b1[None, :], (1, NT))], axis=1
    ).astype(np.float16)
    in_maps = []
    for m in range(NCORES):
        bs = slice(m * BL, (m + 1) * BL)
        evt = ev[bs].transpose(0, 2, 3, 1).reshape(BL, N, C * N)  # [b, j, (c,i)]
        mst = mask[bs, :, :, 0].transpose(0, 2, 1)                # [b, j, i]
        evm = np.concatenate([evt, mst], axis=2).astype(np.float16)
        in_maps.append(
            {
                "evm": np.ascontiguousarray(evm),
                "sT": np.ascontiguousarray(s[bs].transpose(0, 2, 1)).astype(
                    np.float16
                ),
                "w1t": w1t,
                "cst": cst,
            }
        )
    return in_maps


def unshard_output(per_core_outs):
    """list of per-core "out" arrays [BL, H, NT*C*P] fp16 -> full (B, N, 3, H)."""
    parts = []
    for o in per_core_outs:
        o = o.astype(np.float32).reshape(BL, H, NT, C, P).transpose(0, 2, 4, 3, 1)
        parts.append(np.ascontiguousarray(o).reshape(BL, N, C, H))
    return np.concatenate(parts, axis=0)


_executor = None


def _get_executor():
    """Build the sharded PJRT executable once; reuse across kernel() calls."""
    global _executor
    if _executor is not None:
        return _executor
    import jax
    from jax.sharding import Mesh, PartitionSpec
    from jax.experimental.shard_map import shard_map

    from concourse import bass2jax

    bass2jax.install_neuronx_cc_hook()
    nc = _get_nc()
    partition_name = nc.partition_id_tensor.name if nc.partition_id_tensor else None
    in_names, out_names, out_avals, zero_outs = [], [], [], []
    for alloc in nc.m.functions[0].allocations:
        if not isinstance(alloc, mybir.MemoryLocationSet):
            continue
        name = alloc.memorylocations[0].name
        if alloc.kind == "ExternalInput":
            if name != partition_name:
                in_names.append(name)
        elif alloc.kind == "ExternalOutput":
            out_names.append(name)
            shape = tuple(alloc.tensor_shape)
            dtype = mybir.dt.np(alloc.dtype)
            out_avals.append(jax.core.ShapedArray(shape, dtype))
            zero_outs.append(np.zeros(shape, dtype))
    n_params = len(in_names)
    all_in_names = list(in_names) + list(out_names)
    if partition_name is not None:
        all_in_names.append(partition_name)

    def _body(*args):
        operands = list(args)
        if partition_name is not None:
            operands.append(bass2jax.partition_id_tensor())
        outs = bass2jax._bass_exec_p.bind(
            *operands,
            out_avals=tuple(out_avals),
            in_names=tuple(all_in_names),
            out_names=tuple(out_names),
            lowering_input_output_aliases=(),
            sim_require_finite=True,
            sim_require_nnan=True,
            nc=nc,
        )
        return tuple(outs)

    devices = jax.devices()[:NCORES]
    mesh = Mesh(np.asarray(devices), ("core",))
    donate = tuple(range(n_params, n_params + len(out_names)))
    fn = jax.jit(
        shard_map(
            _body,
            mesh=mesh,
            in_specs=(PartitionSpec("core"),) * (n_params + len(out_names)),
            out_specs=(PartitionSpec("core"),) * len(out_names),
            check_rep=False,
        ),
        donate_argnums=donate,
        keep_unused=True,
    )
    _executor = (fn, in_names, out_names, out_avals, zero_outs)
    return _executor


def kernel(s, ev, mask, W1, b1):
    fn, in_names, out_names, out_avals, zero_outs = _get_executor()
    in_maps = shard_inputs(s, ev, mask, W1, b1)
    concat_in = [
        np.concatenate([in_maps[c][nm] for c in range(NCORES)], axis=0)
        for nm in in_names
    ]
    concat_zeros = [
        np.zeros((NCORES * z.shape[0], *z.shape[1:]), z.dtype) for z in zero_outs
    ]
    out_arrs = fn(*concat_in, *concat_zeros)
    i = out_names.index("out")
    o = np.asarray(out_arrs[i]).reshape(NCORES, *out_avals[i].shape)
    return unshard_output([o[c] for c in range(NCORES)])
